# revision 14
# baseline (speedup 1.0000x reference)
"""Trainium2 Bass kernel for nn_BlockedMLP (dense_mlp, 8 cores).

Strategy:
  - 8-way data parallel over the batch (B=2048 -> 256 rows/core), weights
    replicated. No collectives.
  - The BSR fc2 (50% block density, 32x32 blocks) is scattered into a dense
    [H, H] matrix on the host: on the PE array a matmul costs N streamed
    columns regardless of contraction K, so 32x32 sparse blocks waste ~4x
    throughput vs dense 128x128 tiles and the block gather costs more than
    the 2x FLOP saving.
  - Feature-major ("transposed") layout throughout: activations live in SBUF
    as [feature_partition, batch_free]; weights are the stationary matmul
    operand, activations stream. Host pre-transposes x and the weights, so
    the device kernel needs no transposes at all.
  - bf16 inputs/weights (host cast) with fp32 PSUM accumulation: 1 cycle/row
    on the PE (fp32 is 4) and half the HBM traffic.
  - Each layer runs as "waves" of 8 output tiles: 8 PSUM banks hold the 8
    accumulators (one accumulation group per bank — a matmul with start=True
    zeroes a whole 2KB zero-region, so groups must not share a bank), the
    k-outer loop streams weight k-tiles [128, 1024] from one packed
    sequential DRAM tensor, and ReLU+bias epilogues run on ScalarE.
"""

import numpy as np
import ml_dtypes

try:
    import concourse.bass as bass  # noqa: F401
except ImportError:
    import sys

    for _p in ("/opt/trn_rl_repo", "/root/.axon_site/_ro/trn_rl_repo"):
        if _p not in sys.path:
            sys.path.insert(0, _p)

import concourse.bacc as bacc
import concourse.bass as bass
import concourse.mybir as mybir
import concourse.tile as tile
from concourse import bass_utils

B, IN, H, OUT, BS = 2048, 1024, 2048, 1024, 32
NCORES = 8
BSH = B // NCORES  # 256 batch rows per core
P = 128
WCOLS = 1024  # streamed weight tile = [P, WCOLS] = 8 output tiles of 128

F32 = mybir.dt.float32
RELU = mybir.ActivationFunctionType.Relu
IDENT = mybir.ActivationFunctionType.Identity

# Wave schedule: (kt, n_out_tiles) per wave; weights packed in this order.
# fc1: 2 waves x 8 k-tiles; fc2: 2 waves x 16; fc3: 1 wave x 16.
NW1, NW2, NW3 = 2, 2, 1
KT1, KT2, KT3 = IN // P, H // P, H // P
WSEQ_TILES = NW1 * KT1 + NW2 * KT2 + NW3 * KT3  # 64

_CACHE = {}


def _emit(tc, DT):
    nc = tc.nc

    xT = nc.dram_tensor("xT", [P, KT1, BSH], DT, kind="ExternalInput").ap()
    wseq = nc.dram_tensor("wseq", [WSEQ_TILES, P, WCOLS], DT, kind="ExternalInput").ap()
    bc = nc.dram_tensor("bc", [P, 2 * H // P + OUT // P], F32, kind="ExternalInput").ap()
    outT = nc.dram_tensor("outT", [OUT // P, P, BSH], F32, kind="ExternalOutput").ap()

    from contextlib import ExitStack

    with ExitStack() as ctx:
        wp = ctx.enter_context(tc.tile_pool(name="wpool", bufs=10))
        act = ctx.enter_context(tc.tile_pool(name="act", bufs=1))
        pp = ctx.enter_context(tc.tile_pool(name="ps", bufs=1, space="PSUM"))
        iop = ctx.enter_context(tc.tile_pool(name="io", bufs=1))

        # x + biases load on the Scalar HWDGE queue so the first weight tile
        # streams immediately on an empty Sync queue.
        xt = iop.tile([P, KT1, BSH], DT, tag="x", name="xt")
        nc.scalar.dma_start(xt[:], xT[:])
        xts = [xt[:, k, :] for k in range(KT1)]
        bs = iop.tile([P, 2 * H // P + OUT // P], F32, tag="bs", name="bs")
        nc.scalar.dma_start(bs[:], bc[:])
        b1s = bs[:, 0 : H // P]
        b2s = bs[:, H // P : 2 * H // P]
        b3s = bs[:, 2 * H // P :]

        wslot = [0]  # next tile index in wseq
        # Stripe weight-tile DMAs across independent per-engine HWDGE queues
        # so one queue's slot-semaphore wait doesn't idle all 16 DMA engines.
        dmaq = [nc.sync, nc.scalar]

        def wave(kt, rhs_tiles, bias, bias_off, func, out_dt, tag):
            """8 out tiles [P, BSH] = func(sum_k w_k.T @ rhs_k + bias)."""
            ps = [
                pp.tile([P, BSH], F32, tag=f"ps{i}", name=f"{tag}ps{i}")
                for i in range(WCOLS // P)
            ]
            for k in range(kt):
                w = wp.tile([P, WCOLS], DT, tag="w", name=f"{tag}w{k}")
                # Each weight tile streams as two half-DMAs, one per HWDGE
                # queue: halves the first-tile arrival latency and keeps both
                # queues uniformly loaded.
                hw = WCOLS // 2
                e0 = dmaq[wslot[0] % 2]
                e1 = dmaq[(wslot[0] + 1) % 2]
                e0.dma_start(w[:, 0:hw], wseq[wslot[0]][:, 0:hw])
                e1.dma_start(w[:, hw:WCOLS], wseq[wslot[0]][:, hw:WCOLS])
                wslot[0] += 1
                for j in range(WCOLS // P):
                    nc.tensor.matmul(
                        ps[j][:],
                        w[:, j * P : (j + 1) * P],
                        rhs_tiles[k],
                        start=(k == 0),
                        stop=(k == kt - 1),
                    )
            outs = []
            for j in range(WCOLS // P):
                o = act.tile([P, BSH], out_dt, tag=f"{tag}o{j}", name=f"{tag}o{j}")
                bias_ap = bias[:, bias_off + j : bias_off + j + 1]
                if j % 2 == 0:
                    # Even tiles on ScalarE...
                    nc.scalar.activation(o[:], ps[j][:], func, bias=bias_ap)
                elif func is RELU:
                    # ...odd tiles on the otherwise-idle VectorE.
                    nc.vector.tensor_scalar(
                        o[:], ps[j][:], bias_ap, 0.0, mybir.AluOpType.add,
                        mybir.AluOpType.max,
                    )
                else:
                    nc.vector.tensor_scalar_add(o[:], ps[j][:], bias_ap)
                outs.append(o[:])
            return outs

        hts = []
        for wv in range(NW1):
            hts += wave(KT1, xts, b1s, wv * 8, RELU, DT, f"l1w{wv}")
        h2s = []
        for wv in range(NW2):
            h2s += wave(KT2, hts, b2s, wv * 8, RELU, DT, f"l2w{wv}")
        os_ = wave(KT3, h2s, b3s, 0, IDENT, F32, "l3w0")

        for j in range(OUT // P):
            dmaq[j % len(dmaq)].dma_start(outT[j], os_[j])


def _build(dt_name):
    if dt_name in _CACHE:
        return _CACHE[dt_name]
    DT = {"bf16": mybir.dt.bfloat16, "f32r": mybir.dt.float32r, "f32": F32}[dt_name]
    nc = bacc.Bacc(
        "TRN2",
        target_bir_lowering=False,
        debug=False,
        enable_asserts=False,
        num_devices=NCORES,
    )
    with tile.TileContext(nc) as tc:
        _emit(tc, DT)
    nc.compile()
    _CACHE[dt_name] = nc
    return nc


def _np_dt(dt_name):
    return mybir.dt.np(
        {"bf16": mybir.dt.bfloat16, "f32r": mybir.dt.float32r, "f32": F32}[dt_name]
    )


def _host_prep(x, W1, b1, crow_indices, col_indices, values, b2, W3, b3, npdt):
    rb = crow_indices.shape[0] - 1
    nnz, bs, _ = values.shape
    cb = H // bs
    # Scatter BSR into dense W2 [H, H].
    blocks = np.zeros((rb, cb, bs, bs), np.float32)
    row_ids = (
        np.searchsorted(crow_indices, np.arange(nnz, dtype=np.int64), side="right") - 1
    )
    blocks[row_ids, col_indices] = values
    W2 = blocks.transpose(0, 2, 1, 3).reshape(H, H)

    # Pack the streamed weight sequence: for each layer, for each wave
    # (column-half), the k-tiles [P, WCOLS] in consumption order.
    def waves(wT, kdim, nw):  # wT [kdim, ndim] -> [nw*kt, P, WCOLS]
        kt = kdim // P
        t = wT.reshape(kt, P, nw, WCOLS).astype(npdt)
        return np.ascontiguousarray(t.transpose(2, 0, 1, 3).reshape(nw * kt, P, WCOLS))

    wseq = np.concatenate(
        [
            waves(np.ascontiguousarray(W1.T), IN, NW1),
            waves(np.ascontiguousarray(W2.T), H, NW2),
            waves(np.ascontiguousarray(W3.T), H, NW3),
        ]
    )
    bc = np.ascontiguousarray(
        np.concatenate(
            [
                b1.reshape(H // P, P).T,
                b2.reshape(H // P, P).T,
                b3.reshape(OUT // P, P).T,
            ],
            axis=1,
        ).astype(np.float32)
    )
    # x -> per-core transposed shards, [P, kt, BSH] so one DMA loads all.
    xT_all = np.ascontiguousarray(x.T.astype(npdt))  # [IN, B]
    shards = [
        np.ascontiguousarray(
            xT_all[:, c * BSH : (c + 1) * BSH]
            .reshape(KT1, P, BSH)
            .transpose(1, 0, 2)
        )
        for c in range(NCORES)
    ]
    shared = dict(wseq=wseq, bc=bc)
    return [dict(shared, xT=shards[c]) for c in range(NCORES)]


def kernel(x, W1, b1, crow_indices, col_indices, values, b2, W3, b3, _dt="bf16"):
    nc = _build(_dt)
    in_maps = _host_prep(
        np.asarray(x, np.float32),
        np.asarray(W1, np.float32),
        np.asarray(b1, np.float32),
        np.asarray(crow_indices),
        np.asarray(col_indices),
        np.asarray(values, np.float32),
        np.asarray(b2, np.float32),
        np.asarray(W3, np.float32),
        np.asarray(b3, np.float32),
        _np_dt(_dt),
    )
    res = bass_utils.run_bass_kernel_spmd(nc, in_maps, core_ids=list(range(NCORES)))
    out = np.concatenate(
        [res.results[c]["outT"].reshape(OUT, BSH).T for c in range(NCORES)], axis=0
    )
    return np.ascontiguousarray(out.astype(np.float32))


# revision 19
# speedup vs baseline: 1.2637x; 1.2637x over previous
"""Trainium2 Bass kernel for nn_BlockedMLP (dense_mlp, 8 cores).

Strategy:
  - 8-way data parallel over the batch (B=2048 -> 256 rows/core), weights
    replicated. No collectives.
  - The BSR fc2 (50% block density, 32x32 blocks) is scattered into a dense
    [H, H] matrix on the host: on the PE array a matmul costs N streamed
    columns regardless of contraction K, so 32x32 sparse blocks waste ~4x
    throughput vs dense 128x128 tiles and the block gather costs more than
    the 2x FLOP saving.
  - Feature-major ("transposed") layout throughout: activations live in SBUF
    as [feature_partition, batch_free]; weights are the stationary matmul
    operand, activations stream. Host pre-transposes x and the weights, so
    the device kernel needs no transposes at all.
  - bf16 inputs/weights (host cast) with fp32 PSUM accumulation: 1 cycle/row
    on the PE (fp32 is 4) and half the HBM traffic.
  - Each layer runs as "waves" of 8 output tiles: 8 PSUM banks hold the 8
    accumulators (one accumulation group per bank — a matmul with start=True
    zeroes a whole 2KB zero-region, so groups must not share a bank), the
    k-outer loop streams weight k-tiles [128, 1024] from one packed
    sequential DRAM tensor, and ReLU+bias epilogues run on ScalarE.
"""

import numpy as np
import ml_dtypes

try:
    import concourse.bass as bass  # noqa: F401
except ImportError:
    import sys

    for _p in ("/opt/trn_rl_repo", "/root/.axon_site/_ro/trn_rl_repo"):
        if _p not in sys.path:
            sys.path.insert(0, _p)

import concourse.bacc as bacc
import concourse.bass as bass
import concourse.mybir as mybir
import concourse.tile as tile
from concourse import bass_utils

B, IN, H, OUT, BS = 2048, 1024, 2048, 1024, 32
NCORES = 8
BSH = B // NCORES  # 256 batch rows per core
P = 128
WCOLS = 1024  # streamed weight tile = [P, WCOLS] = 8 output tiles of 128

F32 = mybir.dt.float32
RELU = mybir.ActivationFunctionType.Relu
IDENT = mybir.ActivationFunctionType.Identity

# Wave schedule: (kt, n_out_tiles) per wave; weights packed in this order.
# fc1: 2 waves x 8 k-tiles; fc2: 2 waves x 16; fc3: 1 wave x 16.
NW1, NW2, NW3 = 2, 2, 1
KT1, KT2, KT3 = IN // P, H // P, H // P
WSEQ_TILES = NW1 * KT1 + NW2 * KT2 + NW3 * KT3  # 64

_CACHE = {}


def _emit(tc, DT):
    nc = tc.nc

    xT = nc.dram_tensor("xT", [P, KT1, BSH], DT, kind="ExternalInput").ap()
    wseq = nc.dram_tensor(
        "wseq", [WSEQ_TILES // 2, P, 2 * WCOLS], DT, kind="ExternalInput"
    ).ap()
    bc = nc.dram_tensor("bc", [P, 2 * H // P + OUT // P], F32, kind="ExternalInput").ap()
    outT = nc.dram_tensor("outT", [OUT // P, P, BSH], F32, kind="ExternalOutput").ap()

    from contextlib import ExitStack

    with ExitStack() as ctx:
        wp = ctx.enter_context(tc.tile_pool(name="wpool", bufs=6))
        act = ctx.enter_context(tc.tile_pool(name="act", bufs=1))
        pp = ctx.enter_context(tc.tile_pool(name="ps", bufs=1, space="PSUM"))
        iop = ctx.enter_context(tc.tile_pool(name="io", bufs=1))

        # x + biases load on the Scalar HWDGE queue so the first weight tile
        # streams immediately on an empty Sync queue.
        xt = iop.tile([P, KT1, BSH], DT, tag="x", name="xt")
        nc.scalar.dma_start(xt[:], xT[:])
        xts = [xt[:, k, :] for k in range(KT1)]
        bs = iop.tile([P, 2 * H // P + OUT // P], F32, tag="bs", name="bs")
        nc.scalar.dma_start(bs[:], bc[:])
        b1s = bs[:, 0 : H // P]
        b2s = bs[:, H // P : 2 * H // P]
        b3s = bs[:, 2 * H // P :]

        wslot = [0]  # next tile index in wseq
        # Stripe weight-tile DMAs across independent per-engine HWDGE queues
        # so one queue's slot-semaphore wait doesn't idle all 16 DMA engines.
        dmaq = [nc.sync, nc.scalar]

        def wave(kt, rhs_tiles, bias, bias_off, func, out_dt, tag):
            """8 out tiles [P, BSH] = func(sum_k w_k.T @ rhs_k + bias)."""
            ps = [
                pp.tile([P, BSH], F32, tag=f"ps{i}", name=f"{tag}ps{i}")
                for i in range(WCOLS // P)
            ]
            # Two k-tiles per DMA instruction: a dma_start occupies the
            # issuing engine ~700ns regardless of size, so fewer+bigger wins.
            for k2 in range(0, kt, 2):
                w = wp.tile([P, 2 * WCOLS], DT, tag="w", name=f"{tag}w{k2}")
                dmaq[wslot[0] % 2].dma_start(w[:], wseq[wslot[0]])
                wslot[0] += 1
                for kk in range(2):
                    k = k2 + kk
                    for j in range(WCOLS // P):
                        nc.tensor.matmul(
                            ps[j][:],
                            w[:, kk * WCOLS + j * P : kk * WCOLS + (j + 1) * P],
                            rhs_tiles[k],
                            start=(k == 0),
                            stop=(k == kt - 1),
                        )
            outs = []
            for j in range(WCOLS // P):
                o = act.tile([P, BSH], out_dt, tag=f"{tag}o{j}", name=f"{tag}o{j}")
                bias_ap = bias[:, bias_off + j : bias_off + j + 1]
                nc.scalar.activation(o[:], ps[j][:], func, bias=bias_ap)
                outs.append(o[:])
            return outs

        hts = []
        for wv in range(NW1):
            hts += wave(KT1, xts, b1s, wv * 8, RELU, DT, f"l1w{wv}")
        h2s = []
        for wv in range(NW2):
            h2s += wave(KT2, hts, b2s, wv * 8, RELU, DT, f"l2w{wv}")
        os_ = wave(KT3, h2s, b3s, 0, IDENT, F32, "l3w0")

        for j in range(OUT // P):
            dmaq[j % len(dmaq)].dma_start(outT[j], os_[j])


def _build(dt_name):
    if dt_name in _CACHE:
        return _CACHE[dt_name]
    DT = {"bf16": mybir.dt.bfloat16, "f32r": mybir.dt.float32r, "f32": F32}[dt_name]
    nc = bacc.Bacc(
        "TRN2",
        target_bir_lowering=False,
        debug=False,
        enable_asserts=False,
        num_devices=NCORES,
    )
    with tile.TileContext(nc) as tc:
        _emit(tc, DT)
    nc.compile()
    _CACHE[dt_name] = nc
    return nc


def _np_dt(dt_name):
    return mybir.dt.np(
        {"bf16": mybir.dt.bfloat16, "f32r": mybir.dt.float32r, "f32": F32}[dt_name]
    )


def _host_prep(x, W1, b1, crow_indices, col_indices, values, b2, W3, b3, npdt):
    rb = crow_indices.shape[0] - 1
    nnz, bs, _ = values.shape
    cb = H // bs
    # Scatter BSR into dense W2 [H, H].
    blocks = np.zeros((rb, cb, bs, bs), np.float32)
    row_ids = (
        np.searchsorted(crow_indices, np.arange(nnz, dtype=np.int64), side="right") - 1
    )
    blocks[row_ids, col_indices] = values
    W2 = blocks.transpose(0, 2, 1, 3).reshape(H, H)

    # Pack the streamed weight sequence: for each layer, for each wave
    # (column-half), the k-tiles [P, WCOLS] in consumption order.
    def waves(wT, kdim, nw):  # wT [kdim, ndim] -> [nw*kt, P, WCOLS]
        kt = kdim // P
        t = wT.reshape(kt, P, nw, WCOLS).astype(npdt)
        return np.ascontiguousarray(t.transpose(2, 0, 1, 3).reshape(nw * kt, P, WCOLS))

    wseq = np.concatenate(
        [
            waves(np.ascontiguousarray(W1.T), IN, NW1),
            waves(np.ascontiguousarray(W2.T), H, NW2),
            waves(np.ascontiguousarray(W3.T), H, NW3),
        ]
    )
    # Pair consecutive k-tiles side-by-side: [T, P, WCOLS] -> [T/2, P, 2*WCOLS]
    wseq = np.ascontiguousarray(
        wseq.reshape(WSEQ_TILES // 2, 2, P, WCOLS)
        .transpose(0, 2, 1, 3)
        .reshape(WSEQ_TILES // 2, P, 2 * WCOLS)
    )
    bc = np.ascontiguousarray(
        np.concatenate(
            [
                b1.reshape(H // P, P).T,
                b2.reshape(H // P, P).T,
                b3.reshape(OUT // P, P).T,
            ],
            axis=1,
        ).astype(np.float32)
    )
    # x -> per-core transposed shards, [P, kt, BSH] so one DMA loads all.
    xT_all = np.ascontiguousarray(x.T.astype(npdt))  # [IN, B]
    shards = [
        np.ascontiguousarray(
            xT_all[:, c * BSH : (c + 1) * BSH]
            .reshape(KT1, P, BSH)
            .transpose(1, 0, 2)
        )
        for c in range(NCORES)
    ]
    shared = dict(wseq=wseq, bc=bc)
    return [dict(shared, xT=shards[c]) for c in range(NCORES)]


def kernel(x, W1, b1, crow_indices, col_indices, values, b2, W3, b3, _dt="bf16"):
    nc = _build(_dt)
    in_maps = _host_prep(
        np.asarray(x, np.float32),
        np.asarray(W1, np.float32),
        np.asarray(b1, np.float32),
        np.asarray(crow_indices),
        np.asarray(col_indices),
        np.asarray(values, np.float32),
        np.asarray(b2, np.float32),
        np.asarray(W3, np.float32),
        np.asarray(b3, np.float32),
        _np_dt(_dt),
    )
    res = bass_utils.run_bass_kernel_spmd(nc, in_maps, core_ids=list(range(NCORES)))
    out = np.concatenate(
        [res.results[c]["outT"].reshape(OUT, BSH).T for c in range(NCORES)], axis=0
    )
    return np.ascontiguousarray(out.astype(np.float32))
